# revision 6
# baseline (speedup 1.0000x reference)
"""Trainium2 Bass kernel for nn_AttentionModule (segment attention pooling).

Reference computation (N=2M nodes, D=64 feat, B=4096 graphs, batch sorted):
    seg_sum = segment_sum(x, batch)                  # [B, D]
    mean    = seg_sum / max(counts, 1)
    tg      = tanh(mean @ W)                         # [B, D]
    coef    = sigmoid(sum(x * tg[batch], -1))        # [N]
    out     = segment_sum(coef[:, None] * x, batch)  # [B, D]

Strategy: batch is sorted, so graphs are contiguous runs of rows.  Split the
4096 graphs into 8 groups of 512 (one per core) -> no cross-device reduction.
On the host, pad every graph to a fixed slot of CPG chunks of 128 nodes
(zero rows are harmless in every stage), so the device program is fully
uniform across cores (SPMD) and data-independent.

Per core (all engines, per 128-node chunk):
  pass1: PE matmul  lhsT=x_chunk[128,64], rhs=ones[128,1] -> psum col g
         accumulates seg_sum^T [64, 512graphs] in a single PSUM bank.
  tg:    W-matmul on seg_sum^T, PE-transpose to rows, scale by 1/count
         (per-partition scalar), tanh on ACT.
  pass2: rank-1 matmul ones_row x tg[g] broadcasts tg to all 128 rows;
         DVE mul + reduce -> per-node dots; ACT sigmoid;
         scatter = matmul with lhsT = sigmoid column, rhs = x_chunk
         -> out row accumulated in PSUM.
"""

import sys
import numpy as np

sys.path.insert(0, "/opt/trn_rl_repo")

import ml_dtypes  # noqa: E402
from contextlib import ExitStack  # noqa: E402

import concourse.bass as bass  # noqa: E402
import concourse.bacc as bacc  # noqa: E402
import concourse.tile as tile  # noqa: E402
from concourse import mybir  # noqa: E402
from concourse import bass_utils  # noqa: E402

P = 128          # partitions / nodes per chunk
D = 64           # feature dim
NCORES = 8
BF16 = mybir.dt.bfloat16
F32 = mybir.dt.float32

_PROGRAM_CACHE: dict = {}


def _build_program(n_graphs: int, cpg: int):
    """Build + compile the SPMD program. n_graphs per core, cpg chunks/graph."""
    nc = bacc.Bacc(
        "TRN2",
        target_bir_lowering=False,
        debug=False,
        enable_asserts=False,
        num_devices=NCORES,
    )
    chunks = n_graphs * cpg
    n_blk = n_graphs // P            # 128-graph blocks (4)
    n_flush = n_graphs // 8          # output flushes of 8 graphs (64)

    xg = nc.dram_tensor("xg", [P, chunks * D], BF16, kind="ExternalInput")
    recip = nc.dram_tensor("recip", [P, n_blk], F32, kind="ExternalInput")
    wmat = nc.dram_tensor("wmat", [D, D], F32, kind="ExternalInput")
    out = nc.dram_tensor("out", [n_flush, 8 * D], F32, kind="ExternalOutput")

    with tile.TileContext(nc) as tc:
        with ExitStack() as ctx:
            consts = ctx.enter_context(tc.tile_pool(name="consts", bufs=1))
            small = ctx.enter_context(tc.tile_pool(name="small", bufs=1))

            ones_col = consts.tile([P, 1], BF16)
            nc.vector.memset(ones_col[:], 1.0)
            ones_row = consts.tile([1, P], BF16)
            nc.vector.memset(ones_row[:], 1.0)
            # identity for PE transpose: iota(f - p) == 0
            iota_pj = consts.tile([P, P], mybir.dt.int32)
            nc.gpsimd.iota(iota_pj[:], pattern=[[1, P]], base=0, channel_multiplier=-1)
            ident = consts.tile([P, P], F32)
            nc.vector.tensor_scalar(ident[:], iota_pj[:], 0, None, mybir.AluOpType.is_equal)

            w_sb = small.tile([D, D], F32)
            nc.sync.dma_start(w_sb[:], wmat[:])
            recip_sb = small.tile([P, n_blk], F32)
            nc.sync.dma_start(recip_sb[:], recip[:])

            # ---------------- pass 1: seg_sum^T [64, n_graphs] ----------------
            segT_ps_pool = ctx.enter_context(
                tc.tile_pool(name="segps", bufs=1, space="PSUM")
            )
            # one PSUM bank holds 512 fp32 per partition = n_graphs columns
            seg_ps = segT_ps_pool.tile([D, n_graphs], F32)

            with tc.tile_pool(name="x1", bufs=8) as xpool:
                for g in range(n_graphs):
                    xt = xpool.tile([P, cpg * D], BF16, tag="x1")
                    nc.sync.dma_start(xt[:], xg[:, g * cpg * D:(g + 1) * cpg * D])
                    for k in range(cpg):
                        nc.tensor.matmul(
                            seg_ps[:, g:g + 1],
                            xt[:, k * D:(k + 1) * D],
                            ones_col[:],
                            start=(k == 0),
                            stop=(k == cpg - 1),
                        )

            # ---------------- tg = tanh(mean @ W) rows [128, n_blk*D] --------
            segT_sb = small.tile([D, n_graphs], F32)
            nc.scalar.copy(segT_sb[:], seg_ps[:])
            with tc.tile_pool(name="tgps", bufs=2, space="PSUM") as tgps_pool:
                tgpre_ps = tgps_pool.tile([D, n_graphs], F32, tag="tgpre")
                nc.tensor.matmul(tgpre_ps[:], w_sb[:], segT_sb[:], start=True, stop=True)
                tgpre_sb = small.tile([D, n_graphs], F32)
                nc.scalar.copy(tgpre_sb[:], tgpre_ps[:])

                tg_sb = small.tile([P, n_blk * D], BF16)
                for b in range(n_blk):
                    tp_ps = tgps_pool.tile([P, D], F32, tag="tp")
                    nc.tensor.transpose(
                        tp_ps[:], tgpre_sb[:, b * P:(b + 1) * P], ident[0:D, 0:D]
                    )
                    pre_sb = small.tile([P, D], F32, tag=f"pre{b}")
                    nc.vector.tensor_scalar(
                        pre_sb[:], tp_ps[:], recip_sb[:, b:b + 1], None,
                        mybir.AluOpType.mult,
                    )
                    nc.scalar.activation(
                        tg_sb[:, b * D:(b + 1) * D], pre_sb[:],
                        mybir.ActivationFunctionType.Tanh,
                    )

            # flatten tg rows onto partition 0 so matmul base-partition rules hold
            tgflat = small.tile([1, n_graphs * D], BF16)
            for b in range(n_blk):
                nc.sync.dma_start(
                    tgflat[0:1, b * P * D:(b + 1) * P * D],
                    tg_sb[:, b * D:(b + 1) * D],
                )

            # ---------------- pass 2 ----------------
            with ExitStack() as ctx2:
                xpool2 = ctx2.enter_context(tc.tile_pool(name="x2", bufs=8))
                gpool = ctx2.enter_context(tc.tile_pool(name="gsb", bufs=4))
                gps_pool = ctx2.enter_context(
                    tc.tile_pool(name="gps", bufs=3, space="PSUM")
                )
                ops_pool = ctx2.enter_context(
                    tc.tile_pool(name="ops", bufs=2, space="PSUM")
                )
                cpool = ctx2.enter_context(tc.tile_pool(name="coef", bufs=4))
                orow_pool = ctx2.enter_context(tc.tile_pool(name="orow", bufs=2))

                for f in range(n_flush):
                    out_ps = ops_pool.tile([1, 8 * D], F32, tag="outps")
                    for j in range(8):
                        g = f * 8 + j
                        xt = xpool2.tile([P, cpg * D], BF16, tag="x2")
                        nc.sync.dma_start(xt[:], xg[:, g * cpg * D:(g + 1) * cpg * D])
                        # broadcast tg[g] to all 128 partitions, cpg copies
                        g_ps = gps_pool.tile([P, cpg * D], F32, tag="gps")
                        tg_row = tgflat[0:1, g * D:(g + 1) * D]
                        tg_rep = tg_row.rearrange("o (k d) -> o k d", k=1).broadcast_to(
                            [1, cpg, D]
                        )
                        nc.tensor.matmul(g_ps[:], ones_row[:], tg_rep, start=True, stop=True)
                        g_sb = gpool.tile([P, cpg * D], BF16, tag="gsb")
                        nc.scalar.copy(g_sb[:], g_ps[:])
                        # per-node dot products: mul + reduce over D
                        prod = gpool.tile([P, cpg * D], BF16, tag="prod")
                        nc.vector.tensor_tensor(
                            prod[:], xt[:], g_sb[:], mybir.AluOpType.mult
                        )
                        c_sb = cpool.tile([P, cpg], F32, tag="c")
                        nc.vector.tensor_reduce(
                            c_sb[:],
                            prod[:].rearrange("p (k d) -> p k d", k=cpg),
                            mybir.AxisListType.X,
                            mybir.AluOpType.add,
                        )
                        s_sb = cpool.tile([P, cpg], BF16, tag="s")
                        nc.scalar.activation(
                            s_sb[:], c_sb[:], mybir.ActivationFunctionType.Sigmoid
                        )
                        # scatter: out[g] += sum_t sigmoid * x
                        for k in range(cpg):
                            nc.tensor.matmul(
                                out_ps[0:1, j * D:(j + 1) * D],
                                s_sb[:, k:k + 1],
                                xt[:, k * D:(k + 1) * D],
                                start=(k == 0),
                                stop=(k == cpg - 1),
                            )
                    orow = orow_pool.tile([1, 8 * D], F32, tag="orow")
                    nc.scalar.copy(orow[:], out_ps[:])
                    nc.sync.dma_start(out[f:f + 1, :], orow[:])

    nc.compile()
    return nc


def _prep_inputs(x, batch, weight_matrix, size, cpg, n_graphs):
    """Host-side shard + pad. Returns in_maps list for the 8 cores."""
    B = int(size)
    N = x.shape[0]
    starts = np.searchsorted(batch, np.arange(B + 1)).astype(np.int64)
    counts = np.diff(starts)

    x_bf = np.ascontiguousarray(x, dtype=np.float32).astype(ml_dtypes.bfloat16)
    w32 = np.ascontiguousarray(weight_matrix, dtype=np.float32)

    gpc = B // NCORES  # graphs per core
    assert gpc == n_graphs
    slot = cpg * P     # padded nodes per graph

    # destination slot index for every node
    g_of_node = np.asarray(batch, dtype=np.int64)
    off = np.arange(N, dtype=np.int64) - starts[g_of_node]
    dest = g_of_node * slot + off  # global padded index

    in_maps = []
    n_blk = n_graphs // P
    for c in range(NCORES):
        glo, ghi = c * gpc, (c + 1) * gpc
        nlo, nhi = starts[glo], starts[ghi]
        xpad = np.zeros((gpc * slot, D), dtype=ml_dtypes.bfloat16)
        xpad[dest[nlo:nhi] - glo * slot] = x_bf[nlo:nhi]
        # partition-major layout: [chunks,128,D] -> [128, chunks*D]
        xg_pm = np.ascontiguousarray(
            xpad.reshape(gpc * cpg, P, D).transpose(1, 0, 2).reshape(P, -1)
        )
        rc = 1.0 / np.maximum(counts[glo:ghi].astype(np.float32), 1.0)
        recip_pm = np.ascontiguousarray(rc.reshape(n_blk, P).T)
        in_maps.append({"xg": xg_pm, "recip": recip_pm, "wmat": w32})
    return in_maps, counts


def kernel(x, batch, weight_matrix, size, _return_results=False, _trace=False):
    x = np.asarray(x)
    batch = np.asarray(batch)
    weight_matrix = np.asarray(weight_matrix)
    B = int(size)
    assert B % NCORES == 0
    n_graphs = B // NCORES

    starts = np.searchsorted(batch, np.arange(B + 1))
    max_cnt = int(np.diff(starts).max())
    cpg = max(1, -(-max_cnt // P))  # ceil; expected 5 for the 2M/4096 regime

    key = (n_graphs, cpg)
    if key not in _PROGRAM_CACHE:
        _PROGRAM_CACHE[key] = _build_program(n_graphs, cpg)
    nc = _PROGRAM_CACHE[key]

    in_maps, _ = _prep_inputs(x, batch, weight_matrix, size, cpg, n_graphs)
    res = bass_utils.run_bass_kernel_spmd(
        nc, in_maps, core_ids=list(range(NCORES)), trace=_trace
    )
    outs = []
    for c in range(NCORES):
        o = res.results[c]["out"]  # [n_flush, 8*D]
        outs.append(o.reshape(n_graphs, D))
    full = np.concatenate(outs, axis=0).astype(np.float32)
    if _return_results:
        return full, res
    return full
